# revision 70
# baseline (speedup 1.0000x reference)
"""Two-layer GAT (EnhancedGNN) on 8 Trainium2 NeuronCores.

Strategy (graph/data parallel, per sharding hint):
- Nodes are partitioned contiguously across the 8 cores. Each core owns the
  edges whose *dst* falls in its node range (plus the self-loops of its nodes).
- Per layer, each core computes h = x @ W for its node shard (together with
  the fused attention projections es|ed = x @ (W @ a_blk)), packs [h | 1 | es]
  into a 768B bf16 node-table row, and AllGathers the full table so every
  core can gather arbitrary source rows locally.  ed stays in a small
  SBUF-resident per-core table (only the core's own dst nodes need it).
- Edges are packed into one fixed tile per aligned 128-node dst window
  (window t = local nodes [128t, 128t+128)), TE slots per tile.  Source rows
  are fetched with four 640-idx dma_gather calls on four SWDGE queues (small
  gathers on many queues avoid FIFO back-pressure stalls on the gpsimd
  engine).  Indices are int16, so the gathers read from two overlapping
  table ranges (rows [0, 32768) and [NTAB+1-32768, NTAB+1)); edges whose
  src row falls in the overlap are rebalanced host-side so neither half of
  a window exceeds TE/2 slots.
- ed[dst] per edge: the window's 128 ed rows live at edall[:, t, :] in SBUF
  (partition = local node slot), and a step-matrix matmul (CSR offsets
  compared against an edge-slot iota, times the node-wise ed difference)
  expands them to edge-aligned values -- a cumulative-sum telescope.
- Attention logits are exp(leaky_relu(es_src + ed_dst)) (segment-max is
  skipped: logits are bounded by construction, exp is safe in f32); a
  one-hot [edge, node] mask is built by comparing dst-local ids against an
  iota (pad slots carry id 127.5 so they match no node and vanish), and one
  PSUM-accumulated matmul chain computes both the weighted numerator and
  the softmax denominator.  Window results are written back with direct
  DMAs (no indirection anywhere on the device).
- Layer-2's h-phase is fused into the layer-1 edge loop: each window's x1
  tile stays in SBUF, is PE-transposed and projected immediately.
- Only index preprocessing (sort / pack / pad) happens on the host.
"""

import math
import numpy as np
import ml_dtypes

import concourse.bass as bass
import concourse.bacc as bacc
import concourse.mybir as mybir
import concourse.tile as tile
from concourse import bass_utils

F32 = mybir.dt.float32
BF16 = mybir.dt.bfloat16
F16 = mybir.dt.float16
I32 = mybir.dt.int32
I16 = mybir.dt.int16
AF = mybir.ActivationFunctionType
P = 128

NEG_SLOPE = 0.2
IDXSPAN = 32768          # int16 index reach of one dma_gather


def full_cfg():
    return dict(
        N=50000,       # nodes
        C=8,           # cores
        F=256,         # feature dim (in = out for both layers here)
        H=8,           # heads, layer 1
        D=32,          # per-head dim, layer 1
        TE=2304,       # edge slots per tile (TE/2 per table half)
        NQ=4,          # SWDGE queues (one per sub-gather)
        NCHUNK=7,      # AllGather chunks (49 tiles = 7 x 7)
    )


def derive(cfg):
    c = dict(cfg)
    c["NL"] = c["N"] // c["C"]                       # nodes per core
    c["NLP"] = math.ceil(c["NL"] / P) * P            # padded shard rows
    c["NTAB"] = c["C"] * c["NLP"]                    # gathered table rows
    c["NT"] = c["NLP"] // P                          # tiles = 128-node windows
    c["CH"] = c["TE"] // P                           # 128-edge chunks per tile
    c["HE"] = c["TE"] // 2                           # edge slots per half
    c["QH"] = c["HE"] // P                           # chunks per half
    c["BH"] = c["NTAB"] + 1 - IDXSPAN                # high-range base row
    c["HD"] = c["H"] * c["D"]                        # = F
    c["NRC"] = c["NLP"] // c["NCHUNK"]               # own rows per AG chunk
    # bf16 slots per table row: [h(F) | flag | es(2H f32-packed)] -> 256B mult
    c["WROW"] = math.ceil((c["F"] + 2 + 2 * c["H"]) / 128) * 128
    # int16 metadata cols per tile: idx_lo | idx_hi | dlb(bf16) | stt(f16 x2)
    c["SW"] = c["HE"] // 16                          # idx cols per half
    c["TMW"] = 2 * c["SW"] + c["CH"] + 2
    assert c["HD"] == c["F"]
    assert c["HE"] % P == 0 and (c["HE"] // P) in (9, 10)  # 640+512 or 640+640
    assert c["BH"] < IDXSPAN                         # ranges overlap
    assert c["NRC"] % P == 0                         # chunks hold whole tiles
    return c


# --------------------------------------------------------------------------
# host-side graph preprocessing (indices only)
# --------------------------------------------------------------------------

def preprocess(edge_index, cfg):
    """Build per-core, per-tile index arrays.

    Edge slot l in [0, TE): (p, j) = (l % 128, l // 128); slots [0, HE) are
    the low half (table rows [0, IDXSPAN)), slots [HE, TE) the high half
    (rows [BH, NTAB]).  Edges in the overlap [BH, IDXSPAN) go to whichever
    half has room.  Per-tile int16 metadata row layout (per partition p):
      [0, SW)        idx_lo (wrapped by 16, replicated over 8 gpsimd cores)
      [SW, 2SW)      idx_hi (relative to BH; filler -> IDXSPAN-1 = zero row)
      [2SW, 2SW+CH)  dlb: dst-local node slot per edge slot, bf16
                     (pad slots -> 127.5: matches no node, vanishes)
      [2SW+CH, +2)   stt: f16 CSR offset of each node into each half
    Host returns tmeta pre-transposed to [P, NT, TMW].
    """
    c = cfg
    N, C, TE, NT = c["N"], c["C"], c["TE"], c["NT"]
    NL, NLP, NTAB, CH = c["NL"], c["NLP"], c["NTAB"], c["CH"]
    HE, BH, SW, TMW = c["HE"], c["BH"], c["SW"], c["TMW"]

    src = np.asarray(edge_index[0], dtype=np.int64)
    dst = np.asarray(edge_index[1], dtype=np.int64)
    loop = np.arange(N, dtype=np.int64)
    src = np.concatenate([src, loop])
    dst = np.concatenate([dst, loop])
    # chunk-major table layout: [chunk, core, row-within-chunk] so each
    # AllGather chunk lands contiguously
    NRC = c["NRC"]
    m_all = src // NL
    r_all = src % NL
    srow_all = (r_all // NRC) * (C * NRC) + m_all * NRC + (r_all % NRC)

    def wrap16(idx_lin):  # [HE] linear -> [128, HE//16] wrapped+replicated
        a = np.zeros((16, SW), dtype=np.int16)
        a[np.arange(HE) % 16, np.arange(HE) // 16] = idx_lin
        return np.tile(a, (8, 1))

    out = []
    for m in range(C):
        lo, hi = m * NL, (m + 1) * NL
        sel = (dst >= lo) & (dst < hi)
        s_m, d_m = srow_all[sel], dst[sel]
        order = np.argsort(d_m, kind="stable")
        s_m = s_m[order]
        dloc_all = d_m[order] - lo
        starts_all = np.concatenate(
            [[0], np.cumsum(np.bincount(dloc_all, minlength=NL))])

        tm = np.zeros((NT, P, TMW), dtype=np.int16)
        for t in range(NT):
            a, b = t * P, min((t + 1) * P, NL)
            nn = b - a
            idx1 = np.zeros(HE, dtype=np.int64)            # filler: row 0
            idx2 = np.full(HE, IDXSPAN - 1, np.int64)      # filler: zero row
            dl = np.full(TE, -1, dtype=np.int64)           # -1 -> 127.5 pad
            stt = np.zeros((P, 2), dtype=np.float16)
            rows_w = s_m[starts_all[a]:starts_all[b]]
            flex_cap = HE - (rows_w < BH).sum()   # low slots left for overlap
            flex_used = 0
            pl = ph = 0
            for k in range(nn):
                e0, e1 = starts_all[a + k], starts_all[a + k + 1]
                rows_k = s_m[e0:e1]
                is_lo = rows_k < BH
                is_hi = rows_k >= IDXSPAN
                flex = ~(is_lo | is_hi)
                take_lo = flex & (flex_used + np.cumsum(flex) <= flex_cap)
                flex_used += take_lo.sum()
                go_lo = is_lo | take_lo
                low_k = rows_k[go_lo]
                hi_k = rows_k[~go_lo]
                assert pl + len(low_k) <= HE and ph + len(hi_k) <= HE, \
                    f"core {m} tile {t}: half overflow"
                stt[k, 0] = pl
                stt[k, 1] = ph
                idx1[pl:pl + len(low_k)] = low_k
                dl[pl:pl + len(low_k)] = k
                pl += len(low_k)
                idx2[ph:ph + len(hi_k)] = hi_k - BH
                dl[HE + ph:HE + ph + len(hi_k)] = k
                ph += len(hi_k)
            stt[nn:, 0] = pl
            stt[nn:, 1] = ph
            tm[t, :, 0:SW] = wrap16(idx1)
            tm[t, :, SW:2 * SW] = wrap16(idx2)
            dlb = np.where(dl < 0, 127.5, dl.astype(np.float64)).astype(
                ml_dtypes.bfloat16)
            tm[t, :, 2 * SW:2 * SW + CH] = (
                dlb.reshape(CH, P).T.copy().view(np.int16))
            tm[t, :, 2 * SW + CH:2 * SW + CH + 2] = stt.view(np.int16)
        out.append(dict(tmeta=np.ascontiguousarray(tm.transpose(1, 0, 2))))
    return out


# --------------------------------------------------------------------------
# device kernel
# --------------------------------------------------------------------------

def _h_project(nc, cfg, pools, t, xb, Wsb, hown_ch, edall, H, rows):
    """Shared tail of the h-phase: project xb -> [h|1|es] row + ed column.

    Wsb holds [W | a_src-proj | a_dst-proj] fused: one matmul chain yields
    h (F cols), es (H cols), ed (H cols).  `rows` are three pre-zeroed
    [P, WROW] buffers (flag col pre-baked) rotated manually.
    """
    c = cfg
    F, WROW = c["F"], c["WROW"]
    KC = F // P
    TPC = c["NT"] // c["NCHUNK"]
    ps = pools["ps"]
    r0 = (t % TPC) * P
    hp = ps.tile([P, F + 2 * H], F32, tag="psh")
    for k in range(KC):
        nc.tensor.matmul(out=hp[:], lhsT=xb[:, k, :], rhs=Wsb[:, k, :],
                         start=(k == 0), stop=(k == KC - 1))
    row = rows[t % len(rows)]
    nc.scalar.copy(out=row[:, 0:F], in_=hp[:, 0:F])
    rowf = row[:].bitcast(F32)
    nc.vector.tensor_copy(out=rowf[:, F // 2 + 1:F // 2 + 1 + H],
                          in_=hp[:, F:F + H])
    nc.vector.tensor_copy(out=edall[:, t, :], in_=hp[:, F + H:F + 2 * H])
    nc.sync.dma_start(out=hown_ch[t // TPC][r0:r0 + P, :], in_=row[:])


def _h2_fused(nc, cfg, pools, t, res, Wsb, hown_ch, edall, eye, rows):
    """Layer-2 h-phase for one window: res [128, F] f32 lives in SBUF."""
    c = cfg
    F = c["F"]
    KC = F // P
    sb, ps = pools["sb"], pools["ps"]
    xb = sb.tile([P, KC, P], BF16, tag="ph_xb")
    for k in range(KC):
        tp = ps.tile([P, P], F32, tag="psa")
        nc.tensor.transpose(out=tp[:], in_=res[:, k * P:(k + 1) * P],
                            identity=eye[:])
        nc.vector.tensor_copy(out=xb[:, k, :], in_=tp[:])
    _h_project(nc, cfg, pools, t, xb, Wsb, hown_ch, edall, 1, rows)


def _make_difall(nc, cfg, pools, cp, ldiff, edall, H, tag):
    """One batched backward-difference matmul over all windows' ed columns."""
    c = cfg
    NT = c["NT"]
    ps = pools["ps"]
    difp = ps.tile([P, NT * H], F32, tag="psh")
    nc.tensor.matmul(out=difp[:], lhsT=ldiff[:],
                     rhs=edall[:].rearrange("p t h -> p (t h)"),
                     start=True, stop=True)
    difall = cp.tile([P, NT, H], F16, tag=tag)
    nc.scalar.copy(out=difall[:].rearrange("p t h -> p (t h)"), in_=difp[:])
    return difall


def _edge_layer_tile(nc, cfg, pools, layer, t, htab, difall, tmall, consts):
    """One GAT edge-aggregation window; returns the res [128, F] f32 tile."""
    c = cfg
    F, CH, HE, TE = c["F"], c["CH"], c["HE"], c["TE"]
    H = c["H"] if layer == 1 else 1
    D = F // H
    WROW, QH, SW, TMW, BH, NTAB = (c["WROW"], c["QH"], c["SW"], c["TMW"],
                                   c["BH"], c["NTAB"])
    NCOL = F + H                     # psum cols: numer | denom
    sb, ps = pools["sb"], pools["ps"]
    iota_bf, iota_he = consts["iota_bf"], consts["iota_he"]
    tmallbf = tmall[:].bitcast(BF16)
    tmallf16 = tmall[:].bitcast(F16)

    b16 = t * TMW
    i1 = tmall[:, b16:b16 + SW]
    i2 = tmall[:, b16 + SW:b16 + 2 * SW]
    dlb = tmallbf[:, b16 + 2 * SW:b16 + 2 * SW + CH]
    stt = tmallf16[:, b16 + 2 * SW + CH:b16 + 2 * SW + CH + 2]

    # gather source rows [h | 1 | es]: 640+512 idx per half, four queues
    hg = pools["hgp"].tile([P, CH, WROW], BF16, tag="e_hg")
    HQA = 640
    HQB = HE - HQA
    SQA = HQA // 16
    QA = HQA // P
    nc.gpsimd.dma_gather(out_ap=hg[:, 0:QA, :], in_ap=htab[0:IDXSPAN, :],
                         idxs_ap=i1[:, 0:SQA], num_idxs=HQA, num_idxs_reg=HQA,
                         elem_size=WROW, queue_num=0, single_packet=False)
    nc.gpsimd.dma_gather(out_ap=hg[:, QA:QH, :], in_ap=htab[0:IDXSPAN, :],
                         idxs_ap=i1[:, SQA:SW], num_idxs=HQB, num_idxs_reg=HQB,
                         elem_size=WROW, queue_num=1, single_packet=False)
    nc.gpsimd.dma_gather(out_ap=hg[:, QH:QH + QA, :], in_ap=htab[BH:, :],
                         idxs_ap=i2[:, 0:SQA], num_idxs=HQA, num_idxs_reg=HQA,
                         elem_size=WROW, queue_num=2, single_packet=False)
    nc.gpsimd.dma_gather(out_ap=hg[:, QH + QA:CH, :], in_ap=htab[BH:, :],
                         idxs_ap=i2[:, SQA:SW], num_idxs=HQB, num_idxs_reg=HQB,
                         elem_size=WROW, queue_num=3, single_packet=False)

    # ed[dst] per edge via step-matrix cumulative matmul over resident ed rows
    dif = difall[:, t, :]
    step = sb.tile([P, TE], F16, tag="e_step")
    st3 = step[:].rearrange("p (g e) -> p g e", g=2)
    nc.vector.tensor_tensor(
        out=st3,
        in0=iota_he[:].unsqueeze(1).to_broadcast([P, 2, HE]),
        in1=stt.unsqueeze(2).to_broadcast([P, 2, HE]),
        op=mybir.AluOpType.is_ge)
    sed = ps.tile([P, CH * H], F32, tag="sed")
    for j in range(CH):
        nc.tensor.matmul(out=sed[:, j * H:(j + 1) * H],
                         lhsT=step[:, j * P:(j + 1) * P], rhs=dif,
                         start=True, stop=True)

    # s = es[src] + ed[dst]
    hgf = hg[:].bitcast(F32)
    s = sb.tile([P, CH * H], F32, tag="e_s")
    s3 = s[:].rearrange("p (j h) -> p j h", j=CH)
    nc.vector.tensor_tensor(out=s3, in0=hgf[:, :, F // 2 + 1:F // 2 + 1 + H],
                            in1=sed[:].rearrange("p (j h) -> p j h", j=CH),
                            op=mybir.AluOpType.add)
    e1 = sb.tile([P, CH * H], F32, tag="e_e1")
    e2 = sb.tile([P, CH * H], F32, tag="e_e2")
    nc.scalar.activation(out=e1[:], in_=s[:], func=AF.Exp)
    nc.scalar.activation(out=e2[:], in_=s[:], func=AF.Exp, scale=NEG_SLOPE)

    # one-hot dst mask  [128, CH*128] bf16 (pad slots: dlb=127.5, all-zero row)
    mask = sb.tile([P, CH * P], BF16, tag="e_mask")
    m3 = mask[:].rearrange("p (j k) -> p j k", j=CH)
    nc.vector.tensor_tensor(
        out=m3,
        in0=iota_bf[:].unsqueeze(1).to_broadcast([P, CH, P]),
        in1=dlb.unsqueeze(2).to_broadcast([P, CH, P]),
        op=mybir.AluOpType.is_equal)

    psum = ps.tile([P, NCOL], F32, tag="e_psum")
    if layer == 1:
        mm = sb.tile([P, CH * NCOL], BF16, tag="e_mm")
        mm3 = mm[:].rearrange("p (j c) -> p j c", j=CH)
        nc.vector.tensor_tensor(
            out=mm3[:, :, F:F + H],
            in0=e1[:].rearrange("p (j h) -> p j h", j=CH),
            in1=e2[:].rearrange("p (j h) -> p j h", j=CH),
            op=mybir.AluOpType.max)
        nc.vector.tensor_tensor(
            out=mm3[:, :, 0:F].rearrange("p j (h d) -> p j h d", h=H),
            in0=hg[:, :, 0:F].rearrange("p j (h d) -> p j h d", h=H),
            in1=mm3[:, :, F:F + H].unsqueeze(3).to_broadcast([P, CH, H, D]),
            op=mybir.AluOpType.mult)
        for j in range(CH):
            nc.tensor.matmul(out=psum[:], lhsT=mask[:, j * P:(j + 1) * P],
                             rhs=mm[:, j * NCOL:(j + 1) * NCOL],
                             start=(j == 0), stop=(j == CH - 1))
    else:
        w = sb.tile([P, CH], F32, tag="e_w")
        nc.vector.tensor_tensor(out=w[:], in0=e1[:], in1=e2[:],
                                op=mybir.AluOpType.max)
        maskw = sb.tile([P, CH * P], BF16, tag="e_maskw")
        mw3 = maskw[:].rearrange("p (j k) -> p j k", j=CH)
        nc.vector.tensor_tensor(
            out=mw3, in0=m3,
            in1=w[:].unsqueeze(2).to_broadcast([P, CH, P]),
            op=mybir.AluOpType.mult)
        for j in range(CH):
            nc.tensor.matmul(out=psum[:], lhsT=maskw[:, j * P:(j + 1) * P],
                             rhs=hg[:, j, 0:NCOL],
                             start=(j == 0), stop=(j == CH - 1))

    # epilogue: out = elu(numer/denom)   (biases are zero in this problem)
    rec = sb.tile([P, H], F32, tag="e_rec")
    if t == c["NT"] - 1:
        # pad nodes receive no edges; +eps keeps 1/denom finite (z = 0)
        den = sb.tile([P, H], F32, tag="e_den")
        nc.vector.tensor_scalar(out=den[:], in0=psum[:, F:F + H],
                                scalar1=1e-30, scalar2=None,
                                op0=mybir.AluOpType.add)
        nc.vector.reciprocal(out=rec[:], in_=den[:])
    else:
        nc.vector.reciprocal(out=rec[:], in_=psum[:, F:F + H])
    z = sb.tile([P, F], F32, tag="e_z")
    if H == 1:
        nc.scalar.activation(out=z[:], in_=psum[:, 0:F], func=AF.Copy,
                             scale=rec[:, 0:1])
    else:
        nc.vector.tensor_tensor(
            out=z[:].rearrange("p (h d) -> p h d", h=H),
            in0=psum[:, 0:F].rearrange("p (h d) -> p h d", h=H),
            in1=rec[:].unsqueeze(2).to_broadcast([P, H, D]),
            op=mybir.AluOpType.mult)
    rz = sb.tile([P, F], F32, tag="e_rz")
    nc.scalar.activation(out=rz[:], in_=z[:], func=AF.Relu, scale=-1.0)
    ez = sb.tile([P, F], F32, tag="e_ez")
    nc.scalar.activation(out=ez[:], in_=rz[:], func=AF.Exp, scale=-1.0)
    zr = sb.tile([P, F], F32, tag="e_zr")
    nc.vector.tensor_scalar(out=zr[:], in0=z[:], scalar1=0.0, scalar2=-1.0,
                            op0=mybir.AluOpType.max, op1=mybir.AluOpType.add)
    res = sb.tile([P, F], F32, tag="e_res")
    nc.vector.tensor_tensor(out=res[:], in0=ez[:], in1=zr[:],
                            op=mybir.AluOpType.add)
    return res


def build(cfg):
    c = derive(cfg)
    N, C, F, H = c["N"], c["C"], c["F"], c["H"]
    NL, NLP, NTAB, TE, CH, NT = c["NL"], c["NLP"], c["NTAB"], c["TE"], c["CH"], c["NT"]
    WROW, HE, TMW = c["WROW"], c["HE"], c["TMW"]
    KC = F // P

    nc = bacc.Bacc("TRN2", num_devices=C, num_swdge_queues=c.get("NQ", 4))

    # ---- I/O -------------------------------------------------------------
    xT = nc.dram_tensor("xT", [F, NLP], F32, kind="ExternalInput")
    W1 = nc.dram_tensor("W1c", [F, F + 2 * H], F32, kind="ExternalInput")
    W2 = nc.dram_tensor("W2c", [F, F + 2], F32, kind="ExternalInput")
    tm_d = nc.dram_tensor("tmeta", [P, NT, TMW], I16, kind="ExternalInput")
    out_d = nc.dram_tensor("out", [NLP, F], F32, kind="ExternalOutput")

    # ---- internal DRAM ---------------------------------------------------
    NCHUNK, NRC = c["NCHUNK"], c["NRC"]
    h1own_ch = [nc.dram_tensor(f"h1own{i}", [NRC, WROW], BF16)
                for i in range(NCHUNK)]
    htab1 = nc.dram_tensor("htab1", [NTAB + 1, WROW], BF16, addr_space="Shared")
    h2own_ch = [nc.dram_tensor(f"h2own{i}", [NRC, WROW], BF16)
                for i in range(NCHUNK)]
    htab2 = nc.dram_tensor("htab2", [NTAB + 1, WROW], BF16, addr_space="Shared")

    iota_np = np.tile(np.arange(P, dtype=np.float32), (P, 1)).astype(ml_dtypes.bfloat16)
    iota_c = nc.inline_tensor(iota_np, name="iota_c")
    iota_he_np = np.tile(np.arange(HE, dtype=np.float16), (P, 1))
    iota_he_c = nc.inline_tensor(iota_he_np, name="iota_he_c")

    eye_c = nc.inline_tensor(np.eye(P, dtype=np.float32), name="eye_c")
    ldiff_np = np.eye(P, dtype=np.float32)
    ldiff_np[np.arange(P - 1), np.arange(1, P)] = -1.0   # L[k,k+1] = -1
    ldiff_c = nc.inline_tensor(ldiff_np, name="ldiff_c")

    rg = [list(range(C))]

    with tile.TileContext(nc, num_cores=C) as tc:
        with (
            tc.tile_pool(name="const", bufs=1) as cp,
            tc.tile_pool(name="sb", bufs=3) as sb,
            tc.tile_pool(name="hgp", bufs=4) as hgp,
            tc.tile_pool(name="ps", bufs=2, space="PSUM") as ps,
        ):
            pools = dict(sb=sb, ps=ps, hgp=hgp)
            iota_bf = cp.tile([P, P], BF16)
            nc.sync.dma_start(out=iota_bf[:], in_=iota_c[:, :])
            iota_he = cp.tile([P, HE], F16)
            nc.sync.dma_start(out=iota_he[:], in_=iota_he_c[:, :])
            eye = cp.tile([P, P], F32)
            nc.sync.dma_start(out=eye[:], in_=eye_c[:, :])
            ldiff = cp.tile([P, P], F32)
            nc.sync.dma_start(out=ldiff[:], in_=ldiff_c[:, :])
            tmall = cp.tile([P, NT * TMW], I16)
            nc.sync.dma_start(
                out=tmall[:].rearrange("p (t w) -> p t w", t=NT),
                in_=tm_d[:, :, :])
            edall1 = cp.tile([P, NT, H], F32, tag="edall1")
            edall2 = cp.tile([P, NT, 1], F32, tag="edall2")

            def load_w(dram, n, tag):
                tf = cp.tile([P, KC, n], F32, tag=tag + "f")
                tb = cp.tile([P, KC, n], BF16, tag=tag + "b")
                nc.sync.dma_start(out=tf[:],
                                  in_=dram.rearrange("(k p) n -> p k n", k=KC))
                nc.vector.tensor_copy(out=tb[:], in_=tf[:])
                return tb

            W1sb = load_w(W1, F + 2 * H, "w1")
            W2sb = load_w(W2, F + 2, "w2")

            # zero filler row (high-range gathers only)
            zrow = cp.tile([1, WROW], BF16, tag="zrow")
            nc.vector.memset(zrow[:], 0)
            nc.sync.dma_start(out=htab1[NTAB:NTAB + 1, :], in_=zrow[:])
            nc.sync.dma_start(out=htab2[NTAB:NTAB + 1, :], in_=zrow[:])

            # pre-zeroed [h|1|es] row buffers, flag col baked in
            rows = [cp.tile([P, WROW], BF16, tag=f"row{i}", name=f"row{i}")
                    for i in range(3)]
            for r in rows:
                nc.vector.memset(r[:], 0)
                nc.vector.memset(r[:, F:F + 1], 1.0)

            consts = dict(iota_bf=iota_bf, iota_he=iota_he)
            TPC = NT // NCHUNK
            CRC = C * NRC            # table rows per chunk

            def ag_chunk(own_ch, htab, ch):
                nc.gpsimd.collective_compute(
                    "AllGather", mybir.AluOpType.bypass, replica_groups=rg,
                    ins=[own_ch[ch][:, :]],
                    outs=[htab[ch * CRC:(ch + 1) * CRC, :]])

            # ---- layer 1 h + chunked AllGather -------------------------
            for t in range(NT):
                r0 = t * P
                xt = sb.tile([P, KC, P], F32, tag="ph_x")
                nc.sync.dma_start(
                    out=xt[:],
                    in_=xT.rearrange("(k p) m -> p k m", k=KC)[:, :, r0:r0 + P])
                xb = sb.tile([P, KC, P], BF16, tag="ph_xb")
                nc.vector.tensor_copy(out=xb[:], in_=xt[:])
                _h_project(nc, c, pools, t, xb, W1sb, h1own_ch, edall1, H,
                           rows)
                if t % TPC == TPC - 1:
                    ag_chunk(h1own_ch, htab1, t // TPC)
            difall1 = _make_difall(nc, c, pools, cp, ldiff, edall1, H, "dif1")

            # ---- layer-1 edges fused with layer-2 h-phase --------------
            for t in range(NT):
                res = _edge_layer_tile(nc, c, pools, 1, t, htab1, difall1,
                                       tmall, consts)
                _h2_fused(nc, c, pools, t, res, W2sb, h2own_ch, edall2, eye,
                          rows)
                if t % TPC == TPC - 1:
                    ag_chunk(h2own_ch, htab2, t // TPC)
            difall2 = _make_difall(nc, c, pools, cp, ldiff, edall2, 1, "dif2")

            # ---- layer 2 edges -----------------------------------------
            for t in range(NT):
                res = _edge_layer_tile(nc, c, pools, 2, t, htab2, difall2,
                                       tmall, consts)
                nc.sync.dma_start(out=out_d[t * P:(t + 1) * P, :], in_=res[:])

    if not nc.is_finalized():
        nc.finalize()
    return nc, c


# --------------------------------------------------------------------------
# host wrapper
# --------------------------------------------------------------------------

def make_inputs(inputs, cfg, pre):
    """Build per-core in_maps from the full problem inputs."""
    c = cfg
    N, C, F, H = c["N"], c["C"], c["F"], c["H"]
    NL, NLP = c["NL"], c["NLP"]
    x = np.asarray(inputs["x"], dtype=np.float32)
    W1 = np.asarray(inputs["W1"], dtype=np.float32)
    a_src1 = np.asarray(inputs["a_src1"], dtype=np.float32)
    a_dst1 = np.asarray(inputs["a_dst1"], dtype=np.float32)
    W2 = np.asarray(inputs["W2"], dtype=np.float32)
    a_src2 = np.asarray(inputs["a_src2"], dtype=np.float32)
    a_dst2 = np.asarray(inputs["a_dst2"], dtype=np.float32)

    D = c["D"]
    ablk1 = np.zeros((F, 2 * H), dtype=np.float32)
    for h in range(H):
        ablk1[h * D:(h + 1) * D, h] = a_src1[h]
        ablk1[h * D:(h + 1) * D, H + h] = a_dst1[h]
    W1c = np.concatenate([W1, W1 @ ablk1], axis=1)
    ablk2 = np.stack([a_src2[0], a_dst2[0]], axis=1)
    W2c = np.concatenate([W2, W2 @ ablk2], axis=1)

    in_maps = []
    for m in range(C):
        xs = np.zeros((NLP, F), dtype=np.float32)
        xs[:NL] = x[m * NL:(m + 1) * NL]
        im = dict(
            xT=np.ascontiguousarray(xs.T),
            W1c=np.ascontiguousarray(W1c),
            W2c=np.ascontiguousarray(W2c),
            tmeta=pre[m]["tmeta"],
        )
        in_maps.append(im)
    return in_maps


_BUILD_CACHE = {}


def run_full(inputs, cfg=None, trace=False):
    cfg = cfg or full_cfg()
    c = derive(cfg)
    pre = preprocess(np.asarray(inputs["edge_index"]), c)
    key = tuple(sorted(cfg.items()))
    if key not in _BUILD_CACHE:
        _BUILD_CACHE[key] = build(cfg)
    nc, c = _BUILD_CACHE[key]
    in_maps = make_inputs(inputs, c, pre)
    res = bass_utils.run_bass_kernel_spmd(
        nc, in_maps, core_ids=list(range(c["C"])), trace=trace)
    NL = c["NL"]
    out = np.concatenate([res.results[m]["out"][:NL] for m in range(c["C"])], axis=0)
    return out.astype(np.float32), res


def kernel(**inputs):
    out, _ = run_full(inputs)
    return out


# revision 72
# speedup vs baseline: 1.0407x; 1.0407x over previous
"""Two-layer GAT (EnhancedGNN) on 8 Trainium2 NeuronCores.

Strategy (graph/data parallel, per sharding hint):
- Nodes are partitioned contiguously across the 8 cores. Each core owns the
  edges whose *dst* falls in its node range (plus the self-loops of its nodes).
- Per layer, each core computes h = x @ W for its node shard (together with
  the fused attention projections es|ed = x @ (W @ a_blk)), packs [h | 1 | es]
  into a 768B bf16 node-table row, and AllGathers the full table so every
  core can gather arbitrary source rows locally.  ed stays in a small
  SBUF-resident per-core table (only the core's own dst nodes need it).
- Edges are packed into one fixed tile per aligned 128-node dst window
  (window t = local nodes [128t, 128t+128)), TE slots per tile.  Source rows
  are fetched with four 640-idx dma_gather calls on four SWDGE queues (small
  gathers on many queues avoid FIFO back-pressure stalls on the gpsimd
  engine).  Indices are int16, so the gathers read from two overlapping
  table ranges (rows [0, 32768) and [NTAB+1-32768, NTAB+1)); edges whose
  src row falls in the overlap are rebalanced host-side so neither half of
  a window exceeds TE/2 slots.
- ed[dst] per edge: the window's 128 ed rows live at edall[:, t, :] in SBUF
  (partition = local node slot), and a step-matrix matmul (CSR offsets
  compared against an edge-slot iota, times the node-wise ed difference)
  expands them to edge-aligned values -- a cumulative-sum telescope.
- Attention logits are exp(leaky_relu(es_src + ed_dst)) (segment-max is
  skipped: logits are bounded by construction, exp is safe in f32); a
  one-hot [edge, node] mask is built by comparing dst-local ids against an
  iota (pad slots carry id 127.5 so they match no node and vanish), and one
  PSUM-accumulated matmul chain computes both the weighted numerator and
  the softmax denominator.  Window results are written back with direct
  DMAs (no indirection anywhere on the device).
- Layer-2's h-phase is fused into the layer-1 edge loop: each window's x1
  tile stays in SBUF, is PE-transposed and projected immediately.
- Only index preprocessing (sort / pack / pad) happens on the host.
"""

import math
import numpy as np
import ml_dtypes

import concourse.bass as bass
import concourse.bacc as bacc
import concourse.mybir as mybir
import concourse.tile as tile
from concourse import bass_utils

F32 = mybir.dt.float32
BF16 = mybir.dt.bfloat16
F16 = mybir.dt.float16
I32 = mybir.dt.int32
I16 = mybir.dt.int16
AF = mybir.ActivationFunctionType
P = 128

NEG_SLOPE = 0.2
IDXSPAN = 32768          # int16 index reach of one dma_gather


def full_cfg():
    return dict(
        N=50000,       # nodes
        C=8,           # cores
        F=256,         # feature dim (in = out for both layers here)
        H=8,           # heads, layer 1
        D=32,          # per-head dim, layer 1
        TE=2304,       # edge slots per tile (TE/2 per table half)
        NQ=4,          # SWDGE queues (one per sub-gather)
        NCHUNK=7,      # AllGather chunks (49 tiles = 7 x 7)
    )


def derive(cfg):
    c = dict(cfg)
    c["NL"] = c["N"] // c["C"]                       # nodes per core
    c["NLP"] = math.ceil(c["NL"] / P) * P            # padded shard rows
    c["NTAB"] = c["C"] * c["NLP"]                    # gathered table rows
    c["NT"] = c["NLP"] // P                          # tiles = 128-node windows
    c["CH"] = c["TE"] // P                           # 128-edge chunks per tile
    c["HE"] = c["TE"] // 2                           # edge slots per half
    c["QH"] = c["HE"] // P                           # chunks per half
    c["BH"] = c["NTAB"] + 1 - IDXSPAN                # high-range base row
    c["HD"] = c["H"] * c["D"]                        # = F
    c["NRC"] = c["NLP"] // c["NCHUNK"]               # own rows per AG chunk
    # bf16 slots per table row: [h(F) | flag | es(2H f32-packed)] -> 256B mult
    c["WROW"] = math.ceil((c["F"] + 2 + 2 * c["H"]) / 128) * 128
    # int16 metadata cols per tile: idx_lo | idx_hi | dlb(bf16) | stt(f16 x2)
    c["SW"] = c["HE"] // 16                          # idx cols per half
    c["TMW"] = 2 * c["SW"] + c["CH"] + 2
    assert c["HD"] == c["F"]
    assert c["HE"] % P == 0 and (c["HE"] // P) in (9, 10)  # 640+512 or 640+640
    assert c["BH"] < IDXSPAN                         # ranges overlap
    assert c["NRC"] % P == 0                         # chunks hold whole tiles
    return c


# --------------------------------------------------------------------------
# host-side graph preprocessing (indices only)
# --------------------------------------------------------------------------

def preprocess(edge_index, cfg):
    """Build per-core, per-tile index arrays.

    Edge slot l in [0, TE): (p, j) = (l % 128, l // 128); slots [0, HE) are
    the low half (table rows [0, IDXSPAN)), slots [HE, TE) the high half
    (rows [BH, NTAB]).  Edges in the overlap [BH, IDXSPAN) go to whichever
    half has room.  Per-tile int16 metadata row layout (per partition p):
      [0, SW)        idx_lo (wrapped by 16, replicated over 8 gpsimd cores)
      [SW, 2SW)      idx_hi (relative to BH; filler -> IDXSPAN-1 = zero row)
      [2SW, 2SW+CH)  dlb: dst-local node slot per edge slot, bf16
                     (pad slots -> 127.5: matches no node, vanishes)
      [2SW+CH, +2)   stt: f16 CSR offset of each node into each half
    Host returns tmeta pre-transposed to [P, NT, TMW].
    """
    c = cfg
    N, C, TE, NT = c["N"], c["C"], c["TE"], c["NT"]
    NL, NLP, NTAB, CH = c["NL"], c["NLP"], c["NTAB"], c["CH"]
    HE, BH, SW, TMW = c["HE"], c["BH"], c["SW"], c["TMW"]

    src = np.asarray(edge_index[0], dtype=np.int64)
    dst = np.asarray(edge_index[1], dtype=np.int64)
    loop = np.arange(N, dtype=np.int64)
    src = np.concatenate([src, loop])
    dst = np.concatenate([dst, loop])
    # chunk-major table layout: [chunk, core, row-within-chunk] so each
    # AllGather chunk lands contiguously
    NRC = c["NRC"]
    m_all = src // NL
    r_all = src % NL
    srow_all = (r_all // NRC) * (C * NRC) + m_all * NRC + (r_all % NRC)

    def wrap16(idx_lin):  # [HE] linear -> [128, HE//16] wrapped+replicated
        a = np.zeros((16, SW), dtype=np.int16)
        a[np.arange(HE) % 16, np.arange(HE) // 16] = idx_lin
        return np.tile(a, (8, 1))

    out = []
    for m in range(C):
        lo, hi = m * NL, (m + 1) * NL
        sel = (dst >= lo) & (dst < hi)
        s_m, d_m = srow_all[sel], dst[sel]
        order = np.argsort(d_m, kind="stable")
        s_m = s_m[order]
        dloc_all = d_m[order] - lo
        starts_all = np.concatenate(
            [[0], np.cumsum(np.bincount(dloc_all, minlength=NL))])

        tm = np.zeros((NT, P, TMW), dtype=np.int16)
        for t in range(NT):
            a, b = t * P, min((t + 1) * P, NL)
            nn = b - a
            idx1 = np.zeros(HE, dtype=np.int64)            # filler: row 0
            idx2 = np.full(HE, IDXSPAN - 1, np.int64)      # filler: zero row
            dl = np.full(TE, -1, dtype=np.int64)           # -1 -> 127.5 pad
            stt = np.zeros((P, 2), dtype=np.float16)
            rows_w = s_m[starts_all[a]:starts_all[b]]
            flex_cap = HE - (rows_w < BH).sum()   # low slots left for overlap
            flex_used = 0
            pl = ph = 0
            for k in range(nn):
                e0, e1 = starts_all[a + k], starts_all[a + k + 1]
                rows_k = s_m[e0:e1]
                is_lo = rows_k < BH
                is_hi = rows_k >= IDXSPAN
                flex = ~(is_lo | is_hi)
                take_lo = flex & (flex_used + np.cumsum(flex) <= flex_cap)
                flex_used += take_lo.sum()
                go_lo = is_lo | take_lo
                low_k = rows_k[go_lo]
                hi_k = rows_k[~go_lo]
                assert pl + len(low_k) <= HE and ph + len(hi_k) <= HE, \
                    f"core {m} tile {t}: half overflow"
                stt[k, 0] = pl
                stt[k, 1] = ph
                idx1[pl:pl + len(low_k)] = low_k
                dl[pl:pl + len(low_k)] = k
                pl += len(low_k)
                idx2[ph:ph + len(hi_k)] = hi_k - BH
                dl[HE + ph:HE + ph + len(hi_k)] = k
                ph += len(hi_k)
            stt[nn:, 0] = pl
            stt[nn:, 1] = ph
            tm[t, :, 0:SW] = wrap16(idx1)
            tm[t, :, SW:2 * SW] = wrap16(idx2)
            dlb = np.where(dl < 0, 127.5, dl.astype(np.float64)).astype(
                ml_dtypes.bfloat16)
            tm[t, :, 2 * SW:2 * SW + CH] = (
                dlb.reshape(CH, P).T.copy().view(np.int16))
            tm[t, :, 2 * SW + CH:2 * SW + CH + 2] = stt.view(np.int16)
        out.append(dict(tmeta=np.ascontiguousarray(tm.transpose(1, 0, 2))))
    return out


# --------------------------------------------------------------------------
# device kernel
# --------------------------------------------------------------------------

def _h_project(nc, cfg, pools, t, xb, Wsb, hown_ch, edall, H, rows):
    """Shared tail of the h-phase: project xb -> [h|1|es] row + ed column.

    Wsb holds [W | a_src-proj | a_dst-proj] fused: one matmul chain yields
    h (F cols), es (H cols), ed (H cols).  `rows` are three pre-zeroed
    [P, WROW] buffers (flag col pre-baked) rotated manually.
    """
    c = cfg
    F, WROW = c["F"], c["WROW"]
    KC = F // P
    TPC = c["NT"] // c["NCHUNK"]
    ps = pools["ps"]
    r0 = (t % TPC) * P
    hp = ps.tile([P, F + 2 * H], F32, tag="psh")
    for k in range(KC):
        nc.tensor.matmul(out=hp[:], lhsT=xb[:, k, :], rhs=Wsb[:, k, :],
                         start=(k == 0), stop=(k == KC - 1))
    row = rows[t % len(rows)]
    nc.scalar.copy(out=row[:, 0:F], in_=hp[:, 0:F])
    rowf = row[:].bitcast(F32)
    nc.vector.tensor_copy(out=rowf[:, F // 2 + 1:F // 2 + 1 + H],
                          in_=hp[:, F:F + H])
    nc.vector.tensor_copy(out=edall[:, t, :], in_=hp[:, F + H:F + 2 * H])
    nc.sync.dma_start(out=hown_ch[t // TPC][r0:r0 + P, :], in_=row[:])


def _h2_fused(nc, cfg, pools, t, res, Wsb, hown_ch, edall, eye, rows):
    """Layer-2 h-phase for one window: res [128, F] f32 lives in SBUF."""
    c = cfg
    F = c["F"]
    KC = F // P
    sb, ps = pools["sb"], pools["ps"]
    xb = sb.tile([P, KC, P], BF16, tag="ph_xb")
    for k in range(KC):
        tp = ps.tile([P, P], F32, tag="psa")
        nc.tensor.transpose(out=tp[:], in_=res[:, k * P:(k + 1) * P],
                            identity=eye[:])
        nc.vector.tensor_copy(out=xb[:, k, :], in_=tp[:])
    _h_project(nc, cfg, pools, t, xb, Wsb, hown_ch, edall, 1, rows)


def _make_difall(nc, cfg, pools, cp, ldiff, edall, H, tag):
    """One batched backward-difference matmul over all windows' ed columns."""
    c = cfg
    NT = c["NT"]
    ps = pools["ps"]
    difp = ps.tile([P, NT * H], F32, tag="psh")
    nc.tensor.matmul(out=difp[:], lhsT=ldiff[:],
                     rhs=edall[:].rearrange("p t h -> p (t h)"),
                     start=True, stop=True)
    difall = cp.tile([P, NT, H], F16, tag=tag)
    nc.scalar.copy(out=difall[:].rearrange("p t h -> p (t h)"), in_=difp[:])
    return difall


def _edge_layer_tile(nc, cfg, pools, layer, t, htab, difall, tmall, consts):
    """One GAT edge-aggregation window; returns the res [128, F] f32 tile."""
    c = cfg
    F, CH, HE, TE = c["F"], c["CH"], c["HE"], c["TE"]
    H = c["H"] if layer == 1 else 1
    D = F // H
    WROW, QH, SW, TMW, BH, NTAB = (c["WROW"], c["QH"], c["SW"], c["TMW"],
                                   c["BH"], c["NTAB"])
    NCOL = F + H                     # psum cols: numer | denom
    sb, ps = pools["sb"], pools["ps"]
    iota_bf, iota_he = consts["iota_bf"], consts["iota_he"]
    tmallbf = tmall[:].bitcast(BF16)
    tmallf16 = tmall[:].bitcast(F16)

    b16 = t * TMW
    i1 = tmall[:, b16:b16 + SW]
    i2 = tmall[:, b16 + SW:b16 + 2 * SW]
    dlb = tmallbf[:, b16 + 2 * SW:b16 + 2 * SW + CH]
    stt = tmallf16[:, b16 + 2 * SW + CH:b16 + 2 * SW + CH + 2]

    # gather source rows [h | 1 | es]: 640+512 idx per half, four queues
    hg = pools["hgp"].tile([P, CH, WROW], BF16, tag="e_hg")
    HQA = 640
    HQB = HE - HQA
    SQA = HQA // 16
    QA = HQA // P
    nc.gpsimd.dma_gather(out_ap=hg[:, 0:QA, :], in_ap=htab[0:IDXSPAN, :],
                         idxs_ap=i1[:, 0:SQA], num_idxs=HQA, num_idxs_reg=HQA,
                         elem_size=WROW, queue_num=0)
    nc.gpsimd.dma_gather(out_ap=hg[:, QA:QH, :], in_ap=htab[0:IDXSPAN, :],
                         idxs_ap=i1[:, SQA:SW], num_idxs=HQB, num_idxs_reg=HQB,
                         elem_size=WROW, queue_num=1)
    nc.gpsimd.dma_gather(out_ap=hg[:, QH:QH + QA, :], in_ap=htab[BH:, :],
                         idxs_ap=i2[:, 0:SQA], num_idxs=HQA, num_idxs_reg=HQA,
                         elem_size=WROW, queue_num=2)
    nc.gpsimd.dma_gather(out_ap=hg[:, QH + QA:CH, :], in_ap=htab[BH:, :],
                         idxs_ap=i2[:, SQA:SW], num_idxs=HQB, num_idxs_reg=HQB,
                         elem_size=WROW, queue_num=3)

    # ed[dst] per edge via step-matrix cumulative matmul over resident ed rows
    dif = difall[:, t, :]
    step = sb.tile([P, TE], F16, tag="e_step")
    st3 = step[:].rearrange("p (g e) -> p g e", g=2)
    nc.vector.tensor_tensor(
        out=st3,
        in0=iota_he[:].unsqueeze(1).to_broadcast([P, 2, HE]),
        in1=stt.unsqueeze(2).to_broadcast([P, 2, HE]),
        op=mybir.AluOpType.is_ge)
    sed = ps.tile([P, CH * H], F32, tag="sed")
    for j in range(CH):
        nc.tensor.matmul(out=sed[:, j * H:(j + 1) * H],
                         lhsT=step[:, j * P:(j + 1) * P], rhs=dif,
                         start=True, stop=True)

    # s = es[src] + ed[dst]
    hgf = hg[:].bitcast(F32)
    s = sb.tile([P, CH * H], F32, tag="e_s")
    s3 = s[:].rearrange("p (j h) -> p j h", j=CH)
    nc.vector.tensor_tensor(out=s3, in0=hgf[:, :, F // 2 + 1:F // 2 + 1 + H],
                            in1=sed[:].rearrange("p (j h) -> p j h", j=CH),
                            op=mybir.AluOpType.add)
    e1 = sb.tile([P, CH * H], F32, tag="e_e1")
    e2 = sb.tile([P, CH * H], F32, tag="e_e2")
    nc.scalar.activation(out=e1[:], in_=s[:], func=AF.Exp)
    nc.scalar.activation(out=e2[:], in_=s[:], func=AF.Exp, scale=NEG_SLOPE)

    # one-hot dst mask  [128, CH*128] bf16 (pad slots: dlb=127.5, all-zero row)
    mask = sb.tile([P, CH * P], BF16, tag="e_mask")
    m3 = mask[:].rearrange("p (j k) -> p j k", j=CH)
    nc.vector.tensor_tensor(
        out=m3,
        in0=iota_bf[:].unsqueeze(1).to_broadcast([P, CH, P]),
        in1=dlb.unsqueeze(2).to_broadcast([P, CH, P]),
        op=mybir.AluOpType.is_equal)

    psum = ps.tile([P, NCOL], F32, tag="e_psum")
    if layer == 1:
        mm = sb.tile([P, CH * NCOL], BF16, tag="e_mm")
        mm3 = mm[:].rearrange("p (j c) -> p j c", j=CH)
        nc.vector.tensor_tensor(
            out=mm3[:, :, F:F + H],
            in0=e1[:].rearrange("p (j h) -> p j h", j=CH),
            in1=e2[:].rearrange("p (j h) -> p j h", j=CH),
            op=mybir.AluOpType.max)
        nc.vector.tensor_tensor(
            out=mm3[:, :, 0:F].rearrange("p j (h d) -> p j h d", h=H),
            in0=hg[:, :, 0:F].rearrange("p j (h d) -> p j h d", h=H),
            in1=mm3[:, :, F:F + H].unsqueeze(3).to_broadcast([P, CH, H, D]),
            op=mybir.AluOpType.mult)
        for j in range(CH):
            nc.tensor.matmul(out=psum[:], lhsT=mask[:, j * P:(j + 1) * P],
                             rhs=mm[:, j * NCOL:(j + 1) * NCOL],
                             start=(j == 0), stop=(j == CH - 1))
    else:
        w = sb.tile([P, CH], F32, tag="e_w")
        nc.vector.tensor_tensor(out=w[:], in0=e1[:], in1=e2[:],
                                op=mybir.AluOpType.max)
        maskw = sb.tile([P, CH * P], BF16, tag="e_maskw")
        mw3 = maskw[:].rearrange("p (j k) -> p j k", j=CH)
        nc.vector.tensor_tensor(
            out=mw3, in0=m3,
            in1=w[:].unsqueeze(2).to_broadcast([P, CH, P]),
            op=mybir.AluOpType.mult)
        for j in range(CH):
            nc.tensor.matmul(out=psum[:], lhsT=maskw[:, j * P:(j + 1) * P],
                             rhs=hg[:, j, 0:NCOL],
                             start=(j == 0), stop=(j == CH - 1))

    # epilogue: out = elu(numer/denom)   (biases are zero in this problem)
    rec = sb.tile([P, H], F32, tag="e_rec")
    if t == c["NT"] - 1:
        # pad nodes receive no edges; +eps keeps 1/denom finite (z = 0)
        den = sb.tile([P, H], F32, tag="e_den")
        nc.vector.tensor_scalar(out=den[:], in0=psum[:, F:F + H],
                                scalar1=1e-30, scalar2=None,
                                op0=mybir.AluOpType.add)
        nc.vector.reciprocal(out=rec[:], in_=den[:])
    else:
        nc.vector.reciprocal(out=rec[:], in_=psum[:, F:F + H])
    z = sb.tile([P, F], F32, tag="e_z")
    if H == 1:
        nc.scalar.activation(out=z[:], in_=psum[:, 0:F], func=AF.Copy,
                             scale=rec[:, 0:1])
    else:
        for hh in range(H):
            nc.scalar.activation(out=z[:, hh * D:(hh + 1) * D],
                                 in_=psum[:, hh * D:(hh + 1) * D],
                                 func=AF.Copy, scale=rec[:, hh:hh + 1])
    rz = sb.tile([P, F], F32, tag="e_rz")
    nc.scalar.activation(out=rz[:], in_=z[:], func=AF.Relu, scale=-1.0)
    ez = sb.tile([P, F], F32, tag="e_ez")
    nc.scalar.activation(out=ez[:], in_=rz[:], func=AF.Exp, scale=-1.0)
    zr = sb.tile([P, F], F32, tag="e_zr")
    nc.vector.tensor_scalar(out=zr[:], in0=z[:], scalar1=0.0, scalar2=-1.0,
                            op0=mybir.AluOpType.max, op1=mybir.AluOpType.add)
    res = sb.tile([P, F], F32, tag="e_res")
    nc.vector.tensor_tensor(out=res[:], in0=ez[:], in1=zr[:],
                            op=mybir.AluOpType.add)
    return res


def build(cfg):
    c = derive(cfg)
    N, C, F, H = c["N"], c["C"], c["F"], c["H"]
    NL, NLP, NTAB, TE, CH, NT = c["NL"], c["NLP"], c["NTAB"], c["TE"], c["CH"], c["NT"]
    WROW, HE, TMW = c["WROW"], c["HE"], c["TMW"]
    KC = F // P

    nc = bacc.Bacc("TRN2", num_devices=C, num_swdge_queues=c.get("NQ", 4))

    # ---- I/O -------------------------------------------------------------
    xT = nc.dram_tensor("xT", [F, NLP], F32, kind="ExternalInput")
    W1 = nc.dram_tensor("W1c", [F, F + 2 * H], F32, kind="ExternalInput")
    W2 = nc.dram_tensor("W2c", [F, F + 2], F32, kind="ExternalInput")
    tm_d = nc.dram_tensor("tmeta", [P, NT, TMW], I16, kind="ExternalInput")
    out_d = nc.dram_tensor("out", [NLP, F], F32, kind="ExternalOutput")

    # ---- internal DRAM ---------------------------------------------------
    NCHUNK, NRC = c["NCHUNK"], c["NRC"]
    h1own_ch = [nc.dram_tensor(f"h1own{i}", [NRC, WROW], BF16)
                for i in range(NCHUNK)]
    htab1 = nc.dram_tensor("htab1", [NTAB + 1, WROW], BF16, addr_space="Shared")
    h2own_ch = [nc.dram_tensor(f"h2own{i}", [NRC, WROW], BF16)
                for i in range(NCHUNK)]
    htab2 = nc.dram_tensor("htab2", [NTAB + 1, WROW], BF16, addr_space="Shared")

    iota_np = np.tile(np.arange(P, dtype=np.float32), (P, 1)).astype(ml_dtypes.bfloat16)
    iota_c = nc.inline_tensor(iota_np, name="iota_c")
    iota_he_np = np.tile(np.arange(HE, dtype=np.float16), (P, 1))
    iota_he_c = nc.inline_tensor(iota_he_np, name="iota_he_c")

    eye_c = nc.inline_tensor(np.eye(P, dtype=np.float32), name="eye_c")
    ldiff_np = np.eye(P, dtype=np.float32)
    ldiff_np[np.arange(P - 1), np.arange(1, P)] = -1.0   # L[k,k+1] = -1
    ldiff_c = nc.inline_tensor(ldiff_np, name="ldiff_c")

    rg = [list(range(C))]

    with tile.TileContext(nc, num_cores=C) as tc:
        with (
            tc.tile_pool(name="const", bufs=1) as cp,
            tc.tile_pool(name="sb", bufs=3) as sb,
            tc.tile_pool(name="hgp", bufs=4) as hgp,
            tc.tile_pool(name="ps", bufs=2, space="PSUM") as ps,
        ):
            pools = dict(sb=sb, ps=ps, hgp=hgp)
            iota_bf = cp.tile([P, P], BF16)
            nc.sync.dma_start(out=iota_bf[:], in_=iota_c[:, :])
            iota_he = cp.tile([P, HE], F16)
            nc.sync.dma_start(out=iota_he[:], in_=iota_he_c[:, :])
            eye = cp.tile([P, P], F32)
            nc.sync.dma_start(out=eye[:], in_=eye_c[:, :])
            ldiff = cp.tile([P, P], F32)
            nc.sync.dma_start(out=ldiff[:], in_=ldiff_c[:, :])
            tmall = cp.tile([P, NT * TMW], I16)
            nc.sync.dma_start(
                out=tmall[:].rearrange("p (t w) -> p t w", t=NT),
                in_=tm_d[:, :, :])
            edall1 = cp.tile([P, NT, H], F32, tag="edall1")
            edall2 = cp.tile([P, NT, 1], F32, tag="edall2")

            def load_w(dram, n, tag):
                tf = cp.tile([P, KC, n], F32, tag=tag + "f")
                tb = cp.tile([P, KC, n], BF16, tag=tag + "b")
                nc.sync.dma_start(out=tf[:],
                                  in_=dram.rearrange("(k p) n -> p k n", k=KC))
                nc.vector.tensor_copy(out=tb[:], in_=tf[:])
                return tb

            W1sb = load_w(W1, F + 2 * H, "w1")
            W2sb = load_w(W2, F + 2, "w2")

            # zero filler row (high-range gathers only)
            zrow = cp.tile([1, WROW], BF16, tag="zrow")
            nc.vector.memset(zrow[:], 0)
            nc.sync.dma_start(out=htab1[NTAB:NTAB + 1, :], in_=zrow[:])
            nc.sync.dma_start(out=htab2[NTAB:NTAB + 1, :], in_=zrow[:])

            # pre-zeroed [h|1|es] row buffers, flag col baked in
            rows = [cp.tile([P, WROW], BF16, tag=f"row{i}", name=f"row{i}")
                    for i in range(3)]
            for r in rows:
                nc.vector.memset(r[:], 0)
                nc.vector.memset(r[:, F:F + 1], 1.0)

            consts = dict(iota_bf=iota_bf, iota_he=iota_he)
            TPC = NT // NCHUNK
            CRC = C * NRC            # table rows per chunk

            def ag_chunk(own_ch, htab, ch):
                nc.gpsimd.collective_compute(
                    "AllGather", mybir.AluOpType.bypass, replica_groups=rg,
                    ins=[own_ch[ch][:, :]],
                    outs=[htab[ch * CRC:(ch + 1) * CRC, :]])

            # ---- layer 1 h + chunked AllGather -------------------------
            for t in range(NT):
                r0 = t * P
                xt = sb.tile([P, KC, P], F32, tag="ph_x")
                nc.sync.dma_start(
                    out=xt[:],
                    in_=xT.rearrange("(k p) m -> p k m", k=KC)[:, :, r0:r0 + P])
                xb = sb.tile([P, KC, P], BF16, tag="ph_xb")
                nc.vector.tensor_copy(out=xb[:], in_=xt[:])
                _h_project(nc, c, pools, t, xb, W1sb, h1own_ch, edall1, H,
                           rows)
                if t % TPC == TPC - 1:
                    ag_chunk(h1own_ch, htab1, t // TPC)
            difall1 = _make_difall(nc, c, pools, cp, ldiff, edall1, H, "dif1")

            # ---- layer-1 edges fused with layer-2 h-phase --------------
            for t in range(NT):
                res = _edge_layer_tile(nc, c, pools, 1, t, htab1, difall1,
                                       tmall, consts)
                _h2_fused(nc, c, pools, t, res, W2sb, h2own_ch, edall2, eye,
                          rows)
                if t % TPC == TPC - 1:
                    ag_chunk(h2own_ch, htab2, t // TPC)
            difall2 = _make_difall(nc, c, pools, cp, ldiff, edall2, 1, "dif2")

            # ---- layer 2 edges -----------------------------------------
            for t in range(NT):
                res = _edge_layer_tile(nc, c, pools, 2, t, htab2, difall2,
                                       tmall, consts)
                nc.sync.dma_start(out=out_d[t * P:(t + 1) * P, :], in_=res[:])

    if not nc.is_finalized():
        nc.finalize()
    return nc, c


# --------------------------------------------------------------------------
# host wrapper
# --------------------------------------------------------------------------

def make_inputs(inputs, cfg, pre):
    """Build per-core in_maps from the full problem inputs."""
    c = cfg
    N, C, F, H = c["N"], c["C"], c["F"], c["H"]
    NL, NLP = c["NL"], c["NLP"]
    x = np.asarray(inputs["x"], dtype=np.float32)
    W1 = np.asarray(inputs["W1"], dtype=np.float32)
    a_src1 = np.asarray(inputs["a_src1"], dtype=np.float32)
    a_dst1 = np.asarray(inputs["a_dst1"], dtype=np.float32)
    W2 = np.asarray(inputs["W2"], dtype=np.float32)
    a_src2 = np.asarray(inputs["a_src2"], dtype=np.float32)
    a_dst2 = np.asarray(inputs["a_dst2"], dtype=np.float32)

    D = c["D"]
    ablk1 = np.zeros((F, 2 * H), dtype=np.float32)
    for h in range(H):
        ablk1[h * D:(h + 1) * D, h] = a_src1[h]
        ablk1[h * D:(h + 1) * D, H + h] = a_dst1[h]
    W1c = np.concatenate([W1, W1 @ ablk1], axis=1)
    ablk2 = np.stack([a_src2[0], a_dst2[0]], axis=1)
    W2c = np.concatenate([W2, W2 @ ablk2], axis=1)

    in_maps = []
    for m in range(C):
        xs = np.zeros((NLP, F), dtype=np.float32)
        xs[:NL] = x[m * NL:(m + 1) * NL]
        im = dict(
            xT=np.ascontiguousarray(xs.T),
            W1c=np.ascontiguousarray(W1c),
            W2c=np.ascontiguousarray(W2c),
            tmeta=pre[m]["tmeta"],
        )
        in_maps.append(im)
    return in_maps


_BUILD_CACHE = {}


def run_full(inputs, cfg=None, trace=False):
    cfg = cfg or full_cfg()
    c = derive(cfg)
    pre = preprocess(np.asarray(inputs["edge_index"]), c)
    key = tuple(sorted(cfg.items()))
    if key not in _BUILD_CACHE:
        _BUILD_CACHE[key] = build(cfg)
    nc, c = _BUILD_CACHE[key]
    in_maps = make_inputs(inputs, c, pre)
    res = bass_utils.run_bass_kernel_spmd(
        nc, in_maps, core_ids=list(range(c["C"])), trace=trace)
    NL = c["NL"]
    out = np.concatenate([res.results[m]["out"][:NL] for m in range(c["C"])], axis=0)
    return out.astype(np.float32), res


def kernel(**inputs):
    out, _ = run_full(inputs)
    return out
